# revision 8
# baseline (speedup 1.0000x reference)
"""
DLI loss kernel for Trainium2 (8 NeuronCores, pure data parallel over batch).

Math
----
The reference computes, per (b, j) window pair:
    logits[b,j,k] = h_last[b,j]@w_h + cterm[b,k] + fc_b
    loss_pair     = LSE_k(logits masked to k in [j+3, len_b)) - logits[b,j,j+3]
The h_last@w_h and fc_b terms are constant in k, so they cancel exactly
between the LSE and the positive logit.  The whole LSTM drops out and

    loss = sum_{b, s in [3, len_b)} [ log(sum_{k=s}^{len_b-1} e^{cterm[b,k]})
                                      - cterm[b,s] ] / sum_b (len_b - 3)
    cterm[b,k] = encoder_output[b,k,:] @ fc_w[0, H:]   (valid region only)

cterm values are O(+-2) so no max-subtraction is needed for a stable exp.

Device pipeline (per core, 16 batch rows)
-----------------------------------------
  - 8 two-row DMAs (all pre-issued on the Sync HWDGE ring, 2KB descriptor
    lines): partition p of pair-chunk holds rows (2b, 2b+1), t in {4p..4p+3}.
  - w + mask ride the Scalar HWDGE ring so they land early without
    perturbing the enc stream.
  - PE: warm-up dummy matmuls (HAM un-throttle) then, per row, 4 fp32r
    transposes -> PSUM (no bf16 pre-cast needed) and a bf16 one-hot matvec
    (row b of cterm[16,512], PSUM-accumulated).
  - The PSUM->SBUF copy of each transposed row casts fp32->bf16 in the same
    instruction; copies are spread across DVE / ACT / GpSimd.
  - Tail: exp (ACT, table pre-warmed: Ln then Exp so Exp is resident),
    masked suffix-sum scan (DVE), u=(S-1)*mask (DVE), dummy Ln retrigger
    (table load overlaps the scan), Ln(x+1) with accumulate (ACT),
    masked-cterm accumulate (DVE), mask-sum (DVE).
  - Out = per-partition [16, 4] partials ([ln_sum, mc_sum, denom, 0]);
    host reduces numer = sum(c0 - c1), denom = sum(c2) across partitions
    and cores.
"""

import numpy as np

import concourse.bacc as bacc
import concourse.bass as bass
import concourse.mybir as mybir
import concourse.tile as tile
from concourse import masks
from concourse._compat import with_exitstack
from concourse.bass_utils import run_bass_kernel_spmd

B, T, E, H = 128, 512, 128, 128
NCORES = 8
BPC = B // NCORES  # batch rows per core
NPAIR = BPC // 2

f32 = mybir.dt.float32
f32r = mybir.dt.float32r
bf16 = mybir.dt.bfloat16
i32 = mybir.dt.int32

N_DUMMY = 10  # PE warm-up matmuls to lift the HAM clock gate


@with_exitstack
def _dli_body(ctx, tc):
    nc = tc.nc

    enc = nc.dram_tensor("enc", [BPC, T, E], f32r, kind="ExternalInput").ap()
    msk = nc.dram_tensor("mask", [BPC, T], i32, kind="ExternalInput").ap()
    wv = nc.dram_tensor("w", [E], f32, kind="ExternalInput").ap()
    out = nc.dram_tensor("out", [BPC, 4], f32, kind="ExternalOutput").ap()

    const_pool = ctx.enter_context(tc.tile_pool(name="const", bufs=1))
    chunk_pool = ctx.enter_context(tc.tile_pool(name="chunk", bufs=NPAIR))
    t4_pool = ctx.enter_context(tc.tile_pool(name="t4", bufs=6))
    tp_psum = ctx.enter_context(tc.tile_pool(name="tp_psum", bufs=4, space="PSUM"))
    ct_psum = ctx.enter_context(tc.tile_pool(name="ct_psum", bufs=1, space="PSUM"))
    dm_psum = ctx.enter_context(tc.tile_pool(name="dm_psum", bufs=1, space="PSUM"))
    sc_pool = ctx.enter_context(tc.tile_pool(name="scan", bufs=1))

    # --- all enc DMAs pre-issued on the Sync HWDGE ring (2 rows per DMA) ---
    chunk_tiles = []
    for p in range(NPAIR):
        chunk = chunk_pool.tile([128, 2 * T], f32r)
        chunk_tiles.append(chunk)
        nc.sync.dma_start(
            chunk[:].rearrange("p (r c e) -> p r c e", r=2, c=4),
            enc[2 * p : 2 * p + 2].rearrange("r (a c) e -> a r c e", c=4),
        )

    # --- w + mask on the Scalar ring (lands ~7.5us, well before first use) ---
    w_sb = const_pool.tile([128, 1], f32)
    nc.scalar.dma_start(w_sb[:, :], wv.rearrange("(p one) -> p one", one=1))
    msk_sb = sc_pool.tile([BPC, T], i32)
    nc.scalar.dma_start(msk_sb[:], msk[:, :])

    # --- PE warm-up: real (non-transpose) matmuls so HAM sees the PE busy ---
    ds = const_pool.tile([128, T], f32r)
    nc.vector.memset(ds[:].bitcast(mybir.dt.uint32), 0)
    dummy_ps = dm_psum.tile([128, T], f32)
    for _ in range(N_DUMMY):
        nc.tensor.matmul(
            dummy_ps[:, :], lhsT=ds[:, 0:128], rhs=ds[:], start=True, stop=True
        )

    ident_f = const_pool.tile([128, 128], f32)
    masks.make_identity(nc, ident_f[:])
    ident = const_pool.tile([128, 128], f32r)
    nc.vector.tensor_copy(ident[:], ident_f[:])

    # warm the ACT tables: Ln first, Exp last => the real Exp needs no table
    # load; the Ln reload is re-triggered right after the real Exp so it
    # overlaps the DVE scan.
    warm = const_pool.tile([BPC, 1], f32)
    nc.vector.memset(warm[:], 0.0)
    nc.scalar.activation(warm[:], warm[:], mybir.ActivationFunctionType.Ln, bias=1.0)
    nc.scalar.activation(warm[:], warm[:], mybir.ActivationFunctionType.Exp)

    # one-hot expanded matvec weights: woh[e, 16*b + m] = w[e] * (m == b)
    woh = const_pool.tile([128, BPC * BPC], bf16)
    nc.vector.memset(woh[:].bitcast(mybir.dt.uint16), 0)
    nc.vector.tensor_copy(woh[:, :: BPC + 1], w_sb[:, 0:1].broadcast_to([128, BPC]))

    # mask -> f32, zero first 3 time steps (window starts need s >= 3)
    maskf = sc_pool.tile([BPC, T], f32)
    nc.vector.tensor_copy(maskf[:], msk_sb[:])
    nc.vector.memset(maskf[:, 0:3], 0.0)
    mask3_rev = maskf[:, ::-1]

    acc = sc_pool.tile([BPC, 4], f32)
    nc.vector.memset(acc[:, 3:4], 0.0)
    # denominator: sum(mask3) = len_b - 3; runs during the stream
    nc.vector.tensor_reduce(
        acc[:, 2:3], maskf[:], axis=mybir.AxisListType.X, op=mybir.AluOpType.add
    )

    # --- main loop: per row, 4 fp32r transposes + cast-copy + bf16 matvec ---
    cterm_ps = ct_psum.tile([BPC, T], f32)
    t4_tiles = [None] * BPC

    def emit_matvec(b):
        nc.tensor.matmul(
            cterm_ps[:, :],
            lhsT=woh[:, BPC * b : BPC * (b + 1)],
            rhs=t4_tiles[b][:],
            start=(b == 0),
            stop=(b == BPC - 1),
        )

    copy_engines = [
        lambda o, i: nc.vector.tensor_copy(o, i),
        lambda o, i: nc.scalar.copy(o, i),
    ]
    for b in range(BPC):
        chunk = chunk_tiles[b // 2]
        r = b % 2
        tp = tp_psum.tile([128, T], f32r)
        src = chunk[:].rearrange("p (r c e) -> p r c e", r=2, c=4)
        for j in range(4):
            nc.tensor.transpose(
                tp[:, 128 * j : 128 * (j + 1)], src[:, r, j], ident[:]
            )
        t4 = t4_pool.tile([128, T], bf16)
        t4_tiles[b] = t4
        copy_engines[b % 2](t4[:], tp[:].bitcast(f32))
        if b >= 2:
            emit_matvec(b - 2)
    emit_matvec(BPC - 2)
    emit_matvec(BPC - 1)

    # un-permute + time-reverse view of the PSUM cterm: element i reads
    # cterm[b, 511 - i].
    cterm_rev = cterm_ps[:, :].rearrange("m (j p) -> m p j", j=4)[:, ::-1, ::-1]

    # E = exp(cterm)   (reversed-time coordinates, fused permute via the AP)
    e_sb = sc_pool.tile([BPC, T], f32)
    nc.scalar.activation(
        e_sb[:].rearrange("m (p j) -> m p j", j=4),
        cterm_rev,
        mybir.ActivationFunctionType.Exp,
    )
    # re-trigger the Ln table load NOW so it overlaps the scan below
    nc.scalar.activation(warm[:], warm[:], mybir.ActivationFunctionType.Ln, bias=1.0)

    # suffix sums with the mask folded in: state = (state + E[i]) * mask3_rev[i]
    s_sb = sc_pool.tile([BPC, T], f32)
    nc.vector.tensor_tensor_scan(
        s_sb[:], e_sb[:], mask3_rev, 0.0, mybir.AluOpType.add, mybir.AluOpType.mult
    )

    # u = (S - 1) * mask3; then ln(u + 1) = log(S) on valid, 0 on invalid
    u_sb = sc_pool.tile([BPC, T], f32)
    nc.vector.scalar_tensor_tensor(
        u_sb[:], s_sb[:], 1.0, mask3_rev,
        mybir.AluOpType.subtract, mybir.AluOpType.mult,
    )
    ln_sb = sc_pool.tile([BPC, T], f32)
    nc.scalar.activation(
        ln_sb[:], u_sb[:], mybir.ActivationFunctionType.Ln,
        bias=1.0, scale=1.0, accum_out=acc[:, 0:1],
    )
    # sum(mask3*cterm): order-free, so read the PSUM cterm unpermuted and the
    # mask through the matching permuted view.
    mc_sb = sc_pool.tile([BPC, T], f32)
    nc.vector.scalar_tensor_tensor(
        mc_sb[:].rearrange("m (j p) -> m j p", j=4),
        cterm_ps[:, :].rearrange("m (j p) -> m j p", j=4),
        0.0,
        maskf[:].rearrange("m (p j) -> m j p", j=4),
        mybir.AluOpType.add, mybir.AluOpType.mult, accum_out=acc[:, 1:2],
    )

    # out: per-partition partials; host computes sum(c0 - c1) / sum(c2)
    nc.sync.dma_start(out[:, :], acc[:])


_CACHED_NC = None


def _get_program():
    global _CACHED_NC
    if _CACHED_NC is None:
        nc = bacc.Bacc(
            "TRN2",
            target_bir_lowering=False,
            debug=False,
            enable_asserts=False,
        )
        with tile.TileContext(nc) as tc:
            _dli_body(tc)
        nc.compile()
        _CACHED_NC = nc
    return _CACHED_NC


def _make_in_maps(inputs):
    enc = np.ascontiguousarray(inputs["encoder_output"], dtype=np.float32)
    mask = np.ascontiguousarray(inputs["mask"], dtype=np.int32)
    w_e = np.ascontiguousarray(np.asarray(inputs["fc_w"], dtype=np.float32)[0, H:])
    return [
        {
            "enc": np.ascontiguousarray(enc[i * BPC : (i + 1) * BPC]),
            "mask": np.ascontiguousarray(mask[i * BPC : (i + 1) * BPC]),
            "w": w_e,
        }
        for i in range(NCORES)
    ]


def _finalize(results):
    numer = 0.0
    denom = 0.0
    for r in results:
        o = np.asarray(r["out"], dtype=np.float64)
        numer += float(np.sum(o[:, 0] - o[:, 1]))
        denom += float(np.sum(o[:, 2]))
    return np.asarray(numer / denom, dtype=np.float32)


def kernel(**inputs) -> np.ndarray:
    nc = _get_program()
    res = run_bass_kernel_spmd(nc, _make_in_maps(inputs), list(range(NCORES)))
    return _finalize(res.results)


# revision 9
# speedup vs baseline: 1.1251x; 1.1251x over previous
"""
DLI loss kernel for Trainium2 (8 NeuronCores, pure data parallel over batch).

Math
----
The reference computes, per (b, j) window pair:
    logits[b,j,k] = h_last[b,j]@w_h + cterm[b,k] + fc_b
    loss_pair     = LSE_k(logits masked to k in [j+3, len_b)) - logits[b,j,j+3]
The h_last@w_h and fc_b terms are constant in k, so they cancel exactly
between the LSE and the positive logit.  The whole LSTM drops out and

    loss = sum_{b, s in [3, len_b)} [ log(sum_{k=s}^{len_b-1} e^{cterm[b,k]})
                                      - cterm[b,s] ] / sum_b (len_b - 3)
    cterm[b,k] = encoder_output[b,k,:] @ fc_w[0, H:]   (valid region only)

cterm values are O(+-2) so no max-subtraction is needed for a stable exp.

Device pipeline (per core, 16 batch rows)
-----------------------------------------
  - enc arrives through SWDGE (gpsimd) cast-DMAs that convert fp32 -> bf16
    in the DMA datapath: no on-chip cast pass at all.  2-row chunks;
    partition p holds rows (2b, 2b+1), t in {4p..4p+3} (2KB HBM lines).
  - w + mask ride the Scalar HWDGE ring; out rides the Sync ring.
  - PE: a few bf16 warm-up matmuls (HAM un-throttle), then per row 4 bf16
    transposes -> PSUM and a bf16 one-hot matvec (row b of cterm[16,512],
    PSUM-accumulated).  Transposes are LDWEIGHTS-bound (~107ns each).
  - PSUM->SBUF copies of transposed rows split across DVE and ACT.
  - Tail: exp (ACT; tables pre-warmed Ln-then-Exp so Exp is resident),
    masked suffix-sum scan (DVE), u=(S-1)*mask (DVE), a 1-element Ln that
    READS the exp output (so the scheduler cannot hoist it) to overlap the
    Ln table load with the scan, Ln(x+1)+accumulate (ACT), masked-cterm
    accumulate (DVE), mask-sum (DVE).
  - Out = per-partition [16, 4] partials ([ln_sum, mc_sum, denom, 0]);
    host reduces numer = sum(c0 - c1), denom = sum(c2) over partitions
    and cores.
"""

import numpy as np

import concourse.bacc as bacc
import concourse.bass as bass
import concourse.mybir as mybir
import concourse.tile as tile
from concourse import masks
from concourse._compat import with_exitstack
from concourse.bass_utils import run_bass_kernel_spmd

B, T, E, H = 128, 512, 128, 128
NCORES = 8
BPC = B // NCORES  # batch rows per core
NPAIR = BPC // 2

f32 = mybir.dt.float32
bf16 = mybir.dt.bfloat16
i32 = mybir.dt.int32

N_DUMMY = 6  # PE warm-up matmuls to lift the HAM clock gate


@with_exitstack
def _dli_body(ctx, tc):
    nc = tc.nc

    enc = nc.dram_tensor("enc", [BPC, T, E], f32, kind="ExternalInput").ap()
    msk = nc.dram_tensor("mask", [BPC, T], i32, kind="ExternalInput").ap()
    wv = nc.dram_tensor("w", [E], f32, kind="ExternalInput").ap()
    out = nc.dram_tensor("out", [BPC, 4], f32, kind="ExternalOutput").ap()

    const_pool = ctx.enter_context(tc.tile_pool(name="const", bufs=1))
    chunk_pool = ctx.enter_context(tc.tile_pool(name="chunk", bufs=NPAIR))
    t4_pool = ctx.enter_context(tc.tile_pool(name="t4", bufs=6))
    tp_psum = ctx.enter_context(tc.tile_pool(name="tp_psum", bufs=4, space="PSUM"))
    ct_psum = ctx.enter_context(tc.tile_pool(name="ct_psum", bufs=1, space="PSUM"))
    dm_psum = ctx.enter_context(tc.tile_pool(name="dm_psum", bufs=1, space="PSUM"))
    sc_pool = ctx.enter_context(tc.tile_pool(name="scan", bufs=1))

    # --- enc via SWDGE cast-DMAs (fp32 HBM -> bf16 SBUF), 2 rows per DMA ---
    chunk_tiles = []
    for p in range(NPAIR):
        chunk = chunk_pool.tile([128, 2 * T], bf16)
        chunk_tiles.append(chunk)
        nc.gpsimd.dma_start(
            chunk[:].rearrange("p (r c e) -> p r c e", r=2, c=4),
            enc[2 * p : 2 * p + 2].rearrange("r (a c) e -> a r c e", c=4),
        )

    # --- w + mask on the Scalar HWDGE ring (lands early, off the enc path) ---
    w_sb = const_pool.tile([128, 1], f32)
    nc.scalar.dma_start(w_sb[:, :], wv.rearrange("(p one) -> p one", one=1))
    msk_sb = sc_pool.tile([BPC, T], i32)
    nc.scalar.dma_start(msk_sb[:], msk[:, :])

    ident = const_pool.tile([128, 128], bf16)
    masks.make_identity(nc, ident[:])

    # --- PE warm-up: real bf16 matmuls so HAM sees the PE busy ---
    ds = const_pool.tile([128, T], bf16)
    nc.vector.memset(ds[:].bitcast(mybir.dt.uint16), 0)
    dummy_ps = dm_psum.tile([128, T], f32)
    for _ in range(N_DUMMY):
        nc.tensor.matmul(
            dummy_ps[:, :], lhsT=ds[:, 0:128], rhs=ds[:], start=True, stop=True
        )

    # warm the ACT tables: Ln first, Exp last => the real Exp needs no table
    # load; the Ln reload is re-triggered right after the real Exp (below).
    warm = const_pool.tile([BPC, 1], f32)
    nc.vector.memset(warm[:], 0.0)
    nc.scalar.activation(warm[:], warm[:], mybir.ActivationFunctionType.Ln, bias=1.0)
    nc.scalar.activation(warm[:], warm[:], mybir.ActivationFunctionType.Exp)

    # one-hot expanded matvec weights: woh[e, 16*b + m] = w[e] * (m == b)
    woh = const_pool.tile([128, BPC * BPC], bf16)
    nc.vector.memset(woh[:].bitcast(mybir.dt.uint16), 0)
    nc.vector.tensor_copy(woh[:, :: BPC + 1], w_sb[:, 0:1].broadcast_to([128, BPC]))

    # mask -> f32, zero first 3 time steps (window starts need s >= 3)
    maskf = sc_pool.tile([BPC, T], f32)
    nc.vector.tensor_copy(maskf[:], msk_sb[:])
    nc.vector.memset(maskf[:, 0:3], 0.0)
    mask3_rev = maskf[:, ::-1]

    acc = sc_pool.tile([BPC, 4], f32)
    nc.vector.memset(acc[:, 3:4], 0.0)
    # denominator: sum(mask3) = len_b - 3; runs during the stream
    nc.vector.tensor_reduce(
        acc[:, 2:3], maskf[:], axis=mybir.AxisListType.X, op=mybir.AluOpType.add
    )

    # --- main loop: per row, 4 bf16 transposes + copy + bf16 matvec ---
    cterm_ps = ct_psum.tile([BPC, T], f32)
    t4_tiles = [None] * BPC

    def emit_matvec(b):
        nc.tensor.matmul(
            cterm_ps[:, :],
            lhsT=woh[:, BPC * b : BPC * (b + 1)],
            rhs=t4_tiles[b][:],
            start=(b == 0),
            stop=(b == BPC - 1),
        )

    copy_engines = [
        lambda o, i: nc.vector.tensor_copy(o, i),
        lambda o, i: nc.scalar.copy(o, i),
    ]
    # DVE gets 10 copies, ACT gets 6 (ACT also owns exp/ln + table loads)
    copy_sel = [0, 1, 0, 0, 1, 0, 0, 1, 0, 1, 0, 0, 1, 0, 1, 0]
    for b in range(BPC):
        chunk = chunk_tiles[b // 2]
        r = b % 2
        tp = tp_psum.tile([128, T], bf16)
        src = chunk[:].rearrange("p (r c e) -> p r c e", r=2, c=4)
        for j in range(4):
            nc.tensor.transpose(
                tp[:, 128 * j : 128 * (j + 1)], src[:, r, j], ident[:]
            )
        t4 = t4_pool.tile([128, T], bf16)
        t4_tiles[b] = t4
        copy_engines[copy_sel[b]](t4[:], tp[:])
        if b >= 2:
            emit_matvec(b - 2)
    emit_matvec(BPC - 2)
    emit_matvec(BPC - 1)

    # un-permute + time-reverse view of the PSUM cterm: element i reads
    # cterm[b, 511 - i].
    cterm_rev = cterm_ps[:, :].rearrange("m (j p) -> m p j", j=4)[:, ::-1, ::-1]

    # E = exp(cterm)   (reversed-time coordinates, fused permute via the AP)
    e_sb = sc_pool.tile([BPC, T], f32)
    nc.scalar.activation(
        e_sb[:].rearrange("m (p j) -> m p j", j=4),
        cterm_rev,
        mybir.ActivationFunctionType.Exp,
    )
    # Re-trigger the Ln table load NOW so it overlaps the scan below.  Reads
    # e_sb so the scheduler cannot hoist it before the exp.
    lnwarm = sc_pool.tile([1, 1], f32)
    nc.scalar.activation(
        lnwarm[:], e_sb[0:1, 0:1], mybir.ActivationFunctionType.Ln, bias=1.0
    )

    # suffix sums with the mask folded in: state = (state + E[i]) * mask3_rev[i]
    s_sb = sc_pool.tile([BPC, T], f32)
    nc.vector.tensor_tensor_scan(
        s_sb[:], e_sb[:], mask3_rev, 0.0, mybir.AluOpType.add, mybir.AluOpType.mult
    )

    # u = (S - 1) * mask3; then ln(u + 1) = log(S) on valid, 0 on invalid
    u_sb = sc_pool.tile([BPC, T], f32)
    nc.vector.scalar_tensor_tensor(
        u_sb[:], s_sb[:], 1.0, mask3_rev,
        mybir.AluOpType.subtract, mybir.AluOpType.mult,
    )
    ln_sb = sc_pool.tile([BPC, T], f32)
    nc.scalar.activation(
        ln_sb[:], u_sb[:], mybir.ActivationFunctionType.Ln,
        bias=1.0, scale=1.0, accum_out=acc[:, 0:1],
    )
    # sum(mask3*cterm): order-free, so read the PSUM cterm unpermuted and the
    # mask through the matching permuted view.
    mc_sb = sc_pool.tile([BPC, T], f32)
    nc.vector.scalar_tensor_tensor(
        mc_sb[:].rearrange("m (j p) -> m j p", j=4),
        cterm_ps[:, :].rearrange("m (j p) -> m j p", j=4),
        0.0,
        maskf[:].rearrange("m (p j) -> m j p", j=4),
        mybir.AluOpType.add, mybir.AluOpType.mult, accum_out=acc[:, 1:2],
    )

    # out: per-partition partials; host computes sum(c0 - c1) / sum(c2)
    nc.sync.dma_start(out[:, :], acc[:])


_CACHED_NC = None


def _get_program():
    global _CACHED_NC
    if _CACHED_NC is None:
        nc = bacc.Bacc(
            "TRN2",
            target_bir_lowering=False,
            debug=False,
            enable_asserts=False,
        )
        with tile.TileContext(nc) as tc:
            _dli_body(tc)
        nc.compile()
        _CACHED_NC = nc
    return _CACHED_NC


def _make_in_maps(inputs):
    enc = np.ascontiguousarray(inputs["encoder_output"], dtype=np.float32)
    mask = np.ascontiguousarray(inputs["mask"], dtype=np.int32)
    w_e = np.ascontiguousarray(np.asarray(inputs["fc_w"], dtype=np.float32)[0, H:])
    return [
        {
            "enc": np.ascontiguousarray(enc[i * BPC : (i + 1) * BPC]),
            "mask": np.ascontiguousarray(mask[i * BPC : (i + 1) * BPC]),
            "w": w_e,
        }
        for i in range(NCORES)
    ]


def _finalize(results):
    numer = 0.0
    denom = 0.0
    for r in results:
        o = np.asarray(r["out"], dtype=np.float64)
        numer += float(np.sum(o[:, 0] - o[:, 1]))
        denom += float(np.sum(o[:, 2]))
    return np.asarray(numer / denom, dtype=np.float32)


def kernel(**inputs) -> np.ndarray:
    nc = _get_program()
    res = run_bass_kernel_spmd(nc, _make_in_maps(inputs), list(range(NCORES)))
    return _finalize(res.results)
